# revision 40
# baseline (speedup 1.0000x reference)
"""Trainium2 Bass kernel for a binarized (1w1a) BasicBlock:

    out1 = hardtanh(BN1(binconv(x, w1)))          # BN in training mode (batch stats)
    out  = hardtanh(BN2(binconv(out1, w2)) + x)   # identity shortcut

binconv(x, w) = conv3x3(sign(x), sign(w), pad=1) * (SCALE / K)

Sharding: data-parallel over batch (4 images per core on 8 cores), weights
replicated.  BN batch statistics (per-channel sum / sum-of-squares) are
exchanged with a tiny cross-core AllGather + local reduce (AllGather has a
~2x lower latency floor than AllReduce on an 8-core ring); a same-sized
dummy AllGather at kernel start absorbs the cold-plan setup cost.

Implementation notes (rewritten from the 349us baseline; ~310-320us):
  - sign() values are exact in fp8; conv3x3 = 9 shifted fp8 DoubleRow
    matmuls accumulating exact integers in fp32 PSUM.  SCALE/K is folded
    into BN eps (eps_eff = eps/(SCALE/K)^2).
  - 4-up packed activations: all 4 local images side by side in one padded
    row of pitch 240 ([z img(56) z] x4 + 8 pad cols, all offsets
    16B-aligned for DoubleRow), so one matmul covers 2 output rows x 4
    images (free dim 480/512, 93.3% useful vs 87.5% for a per-image
    64-pitch layout); each conv is 28 tiles x 2 channel blocks x 9 MMs.
  - The packed buffer is split into FOUR row-range tiles (rows 0-7 / 6-25
    / 22-41 / 38-57).  Tile tracks dependencies as byte intervals, and a
    matmul's rhs interval spans both channel blocks of its buffer, so any
    sign write into the same buffer would serialize against it; with four
    buffers the sign block for range k+1 runs fully under the matmuls of
    range k.  Both convs reuse the same four buffers.
  - PSUM evacuation: VectorE tensor_scalar copies to fp16 y16 tiles
    ([128, n, h, w]: contiguous per-image blocks for the final phase);
    bn_stats takes one 224-element record per (2 tiles, image); a single
    bn_aggr over tiles 0-25 is issued under the conv tail so only the
    last pair's records + aggregation are exposed at each BN boundary.
  - sign2 for conv2 is sign(A1*y1 + B1) fused into one ScalarE activation
    per (buffer, channel block) with per-partition scale/bias.
  - final phase out = clip(A2*y2 + B2 + x, -1, 1) is computed in place in
    the resident fp16 x tiles and is output-DMA-bound (6.4 MB ~ 18us):
    block 0 runs an all-Vector path in half-blocks so DMAs start ~2us
    after A2/B2; ScalarE affines 6 blocks in place in y16; GpSimd
    (pre-warmed mid-conv2) takes 3 middle clips; output DMAs ride the
    sync queue so the gpsimd software-DGE drain stays short.
  - startup: x rows [0:7) of all images land first (one contiguous DMA
    per (image, channel block)), 8 junk matmuls warm the PE HAM clock
    gate while they're in flight, and the first sign covers only rows
    0-2 so the first conv matmul issues at ~15us.
"""

import numpy as np
import ml_dtypes

import concourse.bass as bass
import concourse.tile as tile
from concourse import bacc, mybir
from concourse import bass_utils

N_CORES = 8
N, C, H, W = 32, 256, 56, 56
NL = N // N_CORES          # images per core (4)
CB = C // 128              # channel blocks (2)
HP = H + 2                 # padded rows (58)
IW = 58                    # per-image slot in a packed row (halo+56+halo share)
RP = 240                   # packed row pitch: 4*58 + 8 pad, 16B-aligned
BLK = HP * RP              # per-channel-block packed image elements (13920)
HT = 2                     # output rows per tile
NT = H // HT               # 28 tiles per conv per channel block
FREE = HT * RP             # 480 free-dim per matmul
SCALE, K = 1.0, 2
EPS = 1e-5
ALPHA = SCALE / K
EPS_EFF = EPS / (ALPHA * ALPHA)
M_TOT = float(N * H * W)   # BN reduction count (global batch)

F32 = mybir.dt.float32
F16 = mybir.dt.float16
FP8 = mybir.dt.float8e4
NP_FP8 = ml_dtypes.float8_e4m3
AF = mybir.ActivationFunctionType
ALU = mybir.AluOpType
DR = mybir.MatmulPerfMode.DoubleRow

_CACHE = {}
DEBUG_DUMPS = False


def _packed_views(xb):
    """xb: [128, CB, BLK] fp8 -> per-cib (rows, RP) and (n, rows, 58) views."""
    rows = [xb[:, cib, :].rearrange("p (r c) -> p r c", c=RP) for cib in range(CB)]
    imgs = [r[:, :, 0:NL * IW].rearrange("p r (n c) -> p n r c", c=IW) for r in rows]
    return rows, imgs


def _emit_stats_convert(nc, small, mv_pair, m_loc, st, tagp):
    """(mean, var) pair per cob -> packed (sum, ssum) into st[:, 0:4]."""
    for cob in range(CB):
        mv = mv_pair[cob]
        tmp = small.tile([128, 1], F32, tag=f"cv{tagp}{cob}", name=f"cv{tagp}{cob}")
        nc.vector.tensor_scalar_mul(st[:, 2 * cob:2 * cob + 1], mv[:, 0:1], m_loc)
        nc.vector.tensor_mul(tmp[:], mv[:, 0:1], mv[:, 0:1])
        nc.vector.tensor_add(tmp[:], tmp[:], mv[:, 1:2])
        nc.vector.tensor_scalar_mul(st[:, 2 * cob + 1:2 * cob + 2], tmp[:], m_loc)


def _emit_affine(nc, small, gsum, gb, g_col, epst, a_t, b_t):
    """A = g*rsqrt(var+eps_eff), B = b - mean*A, both cobs batched [128,2].

    gsum: [128, 4] globally-reduced (s0, ss0, s1, ss1)
    gb column layout: g1_0 g1_1 b1_0 b1_1 g2_0 g2_1 b2_0 b2_1
    """
    g4 = gsum[:].rearrange("p (c s) -> p s c", s=2)  # [:,0,:]=sums, [:,1,:]=ssums
    mean2 = small.tile([128, 2], F32, tag="af_mean", name="af_mean")
    ex2 = small.tile([128, 2], F32, tag="af_ex2", name="af_ex2")
    var2 = small.tile([128, 2], F32, tag="af_var", name="af_var")
    rstd = small.tile([128, 2], F32, tag="af_rstd", name="af_rstd")
    nc.vector.tensor_scalar_mul(mean2[:], g4[:, 0, :], 1.0 / M_TOT)
    nc.vector.tensor_scalar_mul(ex2[:], g4[:, 1, :], 1.0 / M_TOT)
    nc.vector.tensor_mul(var2[:], mean2[:], mean2[:])
    nc.vector.tensor_sub(var2[:], ex2[:], var2[:])
    nc.scalar.activation(out=rstd[:], in_=var2[:], func=AF.Sqrt, bias=epst[:])
    nc.vector.reciprocal(rstd[:], rstd[:])
    nc.vector.tensor_mul(a_t[:], gb[:, g_col:g_col + 2], rstd[:])
    nc.vector.tensor_mul(mean2[:], mean2[:], a_t[:])
    nc.vector.tensor_sub(b_t[:], gb[:, g_col + 2:g_col + 4], mean2[:])


def build():
    """Build + compile the per-core Bass program (SPMD, 8 cores)."""
    nc = bacc.Bacc("TRN2", target_bir_lowering=False, debug=False,
                   num_devices=N_CORES)

    x_in = nc.dram_tensor("x16", [NL, C, H, W], F16, kind="ExternalInput").ap()
    w1_in = nc.dram_tensor("w1t", [128, 3, 3, 2, C], FP8, kind="ExternalInput").ap()
    w2_in = nc.dram_tensor("w2t", [128, 3, 3, 2, C], FP8, kind="ExternalInput").ap()
    gb_in = nc.dram_tensor("gb", [128, 8], F32, kind="ExternalInput").ap()
    out_d = nc.dram_tensor("out", [NL, C, H, W], F16, kind="ExternalOutput").ap()
    dbg = {}
    if DEBUG_DUMPS:
        dbg["xb"] = nc.dram_tensor("dbg_xb", [128, CB, BLK], FP8,
                                   kind="ExternalOutput").ap()
        dbg["xb2"] = nc.dram_tensor("dbg_xb2", [128, CB, BLK], FP8,
                                    kind="ExternalOutput").ap()
        for c in range(CB):
            dbg[f"y1_{c}"] = nc.dram_tensor(f"dbg_y1_{c}", [128, NL, H, W], F16,
                                            kind="ExternalOutput").ap()
            dbg[f"y2_{c}"] = nc.dram_tensor(f"dbg_y2_{c}", [128, NL, H, W], F16,
                                            kind="ExternalOutput").ap()
        dbg["st1"] = nc.dram_tensor("dbg_st1", [128, 4], F32,
                                    kind="ExternalOutput").ap()
        dbg["g81"] = nc.dram_tensor("dbg_g81", [128, N_CORES, 4], F32,
                                    kind="ExternalOutput").ap()
        dbg["ab1"] = nc.dram_tensor("dbg_ab1", [128, 4], F32,
                                    kind="ExternalOutput").ap()
        dbg["ab2"] = nc.dram_tensor("dbg_ab2", [128, 4], F32,
                                    kind="ExternalOutput").ap()

    rg = [list(range(N_CORES))]

    with tile.TileContext(nc) as tc:
        import contextlib
        with contextlib.ExitStack() as ctx:
            consts = ctx.enter_context(tc.tile_pool(name="consts", bufs=1))
            xbp = ctx.enter_context(tc.tile_pool(name="xbp", bufs=1))
            y16p = ctx.enter_context(tc.tile_pool(name="y16p", bufs=1))
            xsp = ctx.enter_context(tc.tile_pool(name="xsp", bufs=1))
            statp = ctx.enter_context(tc.tile_pool(name="statp", bufs=1))
            small = ctx.enter_context(tc.tile_pool(name="small", bufs=1))
            psum = ctx.enter_context(tc.tile_pool(name="psum", bufs=8, space="PSUM"))
            dram = ctx.enter_context(tc.tile_pool(name="dram", bufs=1, space="DRAM"))

            # ---- junk-matmul HAM warmup material (emitted first so the PE
            # runs while the first x stripes are still in flight)
            jw = consts.tile([128, 128], FP8, tag="jw", name="jw")
            jx = consts.tile([128, FREE], FP8, tag="jx", name="jx")
            nc.vector.memset(jw[:].bitcast(mybir.dt.uint32), 0)
            nc.vector.memset(jx[:].bitcast(mybir.dt.uint32), 0)
            gs_warm = consts.tile([128, 64], F16, tag="gsw", name="gsw")
            nc.vector.memset(gs_warm[:].bitcast(mybir.dt.uint32), 0)
            jp = psum.tile([128, FREE + 2], F32, tag="pt", name="jp")
            for _ in range(8):
                nc.tensor.matmul(jp[:, 0:FREE], jw[:], jx[:], start=True, stop=True)

            # ---- staged fp16 x per channel block; doubles as the residual
            # and (in place) the final output buffer.
            xs4 = [xsp.tile([128, NL, H, W], F16, tag=f"xs4_{c}", name=f"xs4_{c}")
                   for c in range(CB)]

            def load_x(cib, n, r0, r1, q):
                # contiguous per-image chunks: 128 descriptors, fast desc-gen
                q.dma_start(out=xs4[cib][:, n, r0:r1, :],
                            in_=x_in[n, cib * 128:(cib + 1) * 128, r0:r1, :])

            R_SM, R_MID = 7, 25
            # first stripes rows [0:7) gate the first sign block: w1 + n0/n1
            # on sync, n2/n3 on gpsimd (before the collective plumbing)
            for n in range(NL):
                for cib in range(CB):
                    load_x(cib, n, 0, R_SM, nc.sync)
            w1t = consts.tile([128, 3, 3, 2, C], FP8, tag="w1t", name="w1t")
            nc.sync.dma_start(out=w1t[:], in_=w1_in[:])
            for n in range(NL):
                for cib in range(CB):
                    load_x(cib, n, R_SM, R_MID, nc.sync)
            # tail chunks are enqueued mid-conv1 (pre_tile) so the sign
            # blocks emitted before them don't pick up a false WAR on them

            # ---- packed binarized activations: FOUR row-range buffers so
            # sign writes for range k+1 never alias the buffer the PE is
            # currently reading (Tile tracks deps as byte intervals; one
            # matmul's rhs interval spans both channel blocks of a buffer,
            # so any in-buffer sign write would serialize with it).
            # Padded-row ranges: A 0..7 (tiles 0-2), B 6..25 (tiles 3-11),
            # C 22..41 (tiles 12-19), D 38..57 (tiles 20-27).
            XB_BASE = (0, 6, 22, 38)
            XB_ROWS = (8, 20, 20, 20)
            XB_OF_T = lambda t: 0 if t <= 2 else (1 if t <= 11 else (2 if t <= 19 else 3))
            xbufs = [xbp.tile([128, CB, r * RP], FP8, tag=f"xb{i}", name=f"xb{i}")
                     for i, r in enumerate(XB_ROWS)]

            def memset_halo(bi, q):
                """zero halo cols + pitch padding (+ halo rows in A/D)."""
                rows_b = XB_ROWS[bi]
                for cib in range(CB):
                    rows = xbufs[bi][:, cib, :].rearrange("p (r c) -> p r c", c=RP)
                    q.memset(rows[:, :, NL * IW:RP].bitcast(mybir.dt.uint32), 0)
                    imgs = rows[:, :, 0:NL * IW].rearrange(
                        "p r (n c) -> p r n c", c=IW)
                    q.memset(imgs[:, :, :, 0:1], 0.0)
                    q.memset(imgs[:, :, :, IW - 1:IW], 0.0)
                    if bi == 0:
                        q.memset(rows[:, 0, :], 0.0)
                    if bi == 3:
                        q.memset(rows[:, rows_b - 1, :], 0.0)

            memset_halo(0, nc.vector)
            memset_halo(1, nc.vector)   # B is needed by ~t=27us; vector is fast

            epst = small.tile([128, 1], F32, tag="epst", name="epst")
            nc.vector.memset(epst[:], EPS_EFF)

            # ---- dummy AllGather: absorb first-collective setup cost
            dzero = small.tile([128, 4], F32, tag="dzero", name="dzero")
            nc.vector.memset(dzero[:], 0.0)
            d_in0 = dram.tile([128, 4], F32, tag="d_in0", name="d_in0")
            d_out0 = dram.tile([N_CORES, 128, 4], F32, tag="d_out0", name="d_out0")
            nc.gpsimd.dma_start(out=d_in0[:], in_=dzero[:])
            nc.gpsimd.collective_compute(
                "AllGather", ALU.bypass, replica_groups=rg,
                ins=[d_in0.opt()], outs=[d_out0.opt()],
            )
            for bi in (2, 3):
                memset_halo(bi, nc.gpsimd)

            # ---- remaining constants
            w2t = consts.tile([128, 3, 3, 2, C], FP8, tag="w2t", name="w2t")
            nc.sync.dma_start(out=w2t[:], in_=w2_in[:])
            gb = consts.tile([128, 8], F32, tag="gb", name="gb")
            nc.sync.dma_start(out=gb[:], in_=gb_in[:])

            # ---- conv outputs as exact integers (reused conv1 -> conv2)
            y16 = [y16p.tile([128, NL, H, W], F16, tag=f"y16_{c}", name=f"y16_{c}")
                   for c in range(CB)]

            # one record per (2 tiles, image): 224-elem contiguous runs.
            # record stride padded to 8 so the 6-wide record dim can't be
            # AP-merged away (the BIR verifier wants innermost extent 6)
            recs = [statp.tile([128, NT // 2, NL, 8], F32, tag=f"rec{c}",
                               name=f"rec{c}") for c in range(CB)]

            # sign blocks: per buffer, interior image rows it must carry
            #   A: img 0..6   B: img 5..24   C: img 21..40   D: img 37..55
            SIGN_RANGES = ((0, 7), (5, 25), (21, 41), (37, 56))

            def sign_block(bi, a_t=None, b_t=None, rr=None):
                """sign into buffer bi: conv1 (from xs4) or conv2 (from y16
                with BN affine) depending on a_t."""
                base = XB_BASE[bi]
                r0, r1 = rr if rr is not None else SIGN_RANGES[bi]
                for cib in range(CB):
                    rows = xbufs[bi][:, cib, :].rearrange(
                        "p (r c) -> p r c", c=RP)
                    dst = (rows[:, :, 0:NL * IW]
                           .rearrange("p r (n c) -> p n r c", c=IW)
                           [:, :, 1 + r0 - base:1 + r1 - base, 1:W + 1])
                    if a_t is None:
                        nc.scalar.activation(
                            out=dst, in_=xs4[cib][:, :, r0:r1, :], func=AF.Sign)
                    else:
                        nc.scalar.activation(
                            out=dst, in_=y16[cib][:, :, r0:r1, :], func=AF.Sign,
                            scale=a_t[:, cib:cib + 1], bias=b_t[:, cib:cib + 1])

            def conv(wt, pre_tile, tagp):
                """One binarized conv3x3 over the 4 packed row-range buffers.

                pre_tile: dict tile_idx -> emit callbacks (sign blocks for
                upcoming buffers, lazy DMA enqueues...).
                Returns per-cob (mean, var) tiles for the local batch."""
                mv = [small.tile([128, 2], F32, tag=f"mv{tagp}{c}", name=f"mv{tagp}{c}")
                      for c in range(CB)]
                mvB = [small.tile([128, 2], F32, tag=f"mvB{tagp}{c}",
                                  name=f"mvB{tagp}{c}") for c in range(CB)]
                for t in range(NT):
                    for fn in pre_tile.get(t, ()):
                        fn()
                    if t == NT - 1:
                        for cob in range(CB):
                            rv = recs[cob][:, 0:NT // 2 - 1, :, 0:6].rearrange(
                                "p t n s -> p (t n) s")
                            nc.vector.bn_aggr(out=mv[cob][:], in_=rv)
                    h0 = t * HT
                    bi = XB_OF_T(t)
                    base = XB_BASE[bi]
                    src = xbufs[bi][:]
                    for cob in range(CB):
                        pt = psum.tile([128, FREE + 2], F32, tag="pt", name="pt")
                        k = 0
                        for dy in range(3):
                            off = (h0 + dy - base) * RP
                            for dx in range(3):
                                nc.tensor.matmul(
                                    pt[:, 2 - dx:FREE + 2 - dx],
                                    wt[:, dy, dx, :, cob * 128:(cob + 1) * 128],
                                    src[:, :, off:off + FREE],
                                    start=(k == 0),
                                    stop=(k == 8),
                                    perf_mode=DR,
                                )
                                k += 1
                        # useful output view, iterated (image, row, col).
                        # out pixel w of image n sits at window offset
                        # r*RP + n*IW + w (NOT +1: the dx-shifted windows
                        # already center the 3x3 stencil)
                        ptv = (pt[:, 2:FREE + 2]
                               .rearrange("p (r c) -> p r c", c=RP)[:, :, 0:NL * IW]
                               .rearrange("p r (n c) -> p n r c", c=IW)[:, :, :, 0:W])
                        nc.vector.tensor_scalar_mul(
                            y16[cob][:, :, h0:h0 + HT, :], ptv, 1.0)
                        if t % 2 == 1:
                            # single-record bn_stats per (2 tiles, image)
                            for n in range(NL):
                                nc.vector.bn_stats(
                                    out=recs[cob][:, t // 2, n, 0:6],
                                    in_=y16[cob][:, n, h0 - HT:h0 + HT, :]
                                    .rearrange("p r c -> p (r c)"))
                for cob in range(CB):
                    nc.vector.bn_aggr(out=mvB[cob][:],
                                      in_=recs[cob][:, NT // 2 - 1, :, 0:6])
                return mv, mvB

            def stats_gather(mvs, tagp):
                """Pack local chunked (mean,var) -> (sum, ssum)[128,4],
                AllGather, reduce the 8 cores' contributions locally."""
                mv, mvB = mvs
                stA = small.tile([128, 4], F32, tag=f"stA{tagp}", name=f"stA{tagp}")
                stB = small.tile([128, 4], F32, tag=f"stB{tagp}", name=f"stB{tagp}")
                _emit_stats_convert(nc, small, mv, float(26 * NL * HT * W),
                                    stA, tagp)
                _emit_stats_convert(nc, small, mvB, float(2 * NL * HT * W),
                                    stB, "b" + tagp)
                nc.vector.tensor_add(stA[:], stA[:], stB[:])
                d_in = dram.tile([128, 4], F32, tag=f"din{tagp}", name=f"din{tagp}")
                d_out = dram.tile([N_CORES, 128, 4], F32, tag=f"dou{tagp}",
                                  name=f"dou{tagp}")
                nc.gpsimd.dma_start(out=d_in[:], in_=stA[:])
                nc.gpsimd.collective_compute(
                    "AllGather", ALU.bypass, replica_groups=rg,
                    ins=[d_in.opt()], outs=[d_out.opt()],
                )
                g8 = small.tile([128, N_CORES, 4], F32, tag=f"g8{tagp}",
                                name=f"g8{tagp}")
                nc.gpsimd.dma_start(out=g8[:],
                                    in_=d_out[:].rearrange("r p c -> p r c"))
                t4 = small.tile([128, 4, 4], F32, tag=f"t4{tagp}", name=f"t4{tagp}")
                t2 = small.tile([128, 2, 4], F32, tag=f"t2{tagp}", name=f"t2{tagp}")
                gs = small.tile([128, 4], F32, tag=f"gs{tagp}", name=f"gs{tagp}")
                nc.vector.tensor_add(t4[:], g8[:, 0:4, :], g8[:, 4:8, :])
                nc.vector.tensor_add(t2[:], t4[:, 0:2, :], t4[:, 2:4, :])
                nc.vector.tensor_add(gs[:].rearrange("p (o c) -> p o c", o=1),
                                     t2[:, 0:1, :], t2[:, 1:2, :])
                if DEBUG_DUMPS and tagp == "1":
                    nc.scalar.dma_start(out=dbg["st1"], in_=stA[:])
                    nc.scalar.dma_start(out=dbg["g81"], in_=g8[:])
                return gs

            def tails_enq():
                for n in range(NL):
                    for cib in range(CB):
                        load_x(cib, n, R_MID, H, nc.sync)

            # ================= conv1 =================
            sign_block(0, rr=(0, 3))     # just enough for conv tile 0
            pre1 = {
                1: [lambda: sign_block(0, rr=(3, 7))],
                2: [lambda: sign_block(1)],
                4: [tails_enq],
                6: [lambda: sign_block(2)],
                12: [lambda: sign_block(3)],
            }
            mv1 = conv(w1t, pre1, "1")  # -> (mvA, mvB)
            if DEBUG_DUMPS:
                for c in range(CB):
                    nc.scalar.dma_start(out=dbg[f"y1_{c}"], in_=y16[c][:])
            gs1 = stats_gather(mv1, "1")
            a1 = small.tile([128, 2], F32, tag="a1", name="a1")
            b1 = small.tile([128, 2], F32, tag="b1", name="b1")
            _emit_affine(nc, small, gs1, gb, 0, epst, a1, b1)
            if DEBUG_DUMPS:
                nc.scalar.dma_start(out=dbg["ab1"][:, 0:2], in_=a1[:])
                nc.scalar.dma_start(out=dbg["ab1"][:, 2:4], in_=b1[:])

            # ================= conv2 =================
            sign_block(0, a1, b1, rr=(0, 3))
            pre2 = {
                1: [lambda: sign_block(0, a1, b1, rr=(3, 7))],
                2: [lambda: sign_block(1, a1, b1)],
                5: [lambda: sign_block(2, a1, b1)],
                12: [lambda: sign_block(3, a1, b1)],
                20: [lambda: nc.gpsimd.tensor_scalar(
                    out=gs_warm[:], in0=gs_warm[:],
                    scalar1=1.0, scalar2=-1.0, op0=ALU.min, op1=ALU.max)],
            }
            mv2 = conv(w2t, pre2, "2")
            if DEBUG_DUMPS:
                for c in range(CB):
                    nc.scalar.dma_start(out=dbg[f"y2_{c}"], in_=y16[c][:])
            gs2 = stats_gather(mv2, "2")
            a2 = small.tile([128, 2], F32, tag="a2", name="a2")
            b2 = small.tile([128, 2], F32, tag="b2", name="b2")
            _emit_affine(nc, small, gs2, gb, 4, epst, a2, b2)
            if DEBUG_DUMPS:
                nc.scalar.dma_start(out=dbg["ab2"][:, 0:2], in_=a2[:])
                nc.scalar.dma_start(out=dbg["ab2"][:, 2:4], in_=b2[:])

            # ================= final =================
            # out = clip(A2*y2 + B2 + x, -1, 1), computed in place in xs4.
            # Output-DMA-bound phase (6.4 MB fp16 ~ 18us): block 0 runs the
            # all-Vector path in half-blocks so DMAs start ~2us after A2/B2;
            # ScalarE affines 6 blocks; GpSimd (pre-warmed) takes 3 middle
            # clips; the last block is halved to shorten the tail.
            blocks = [(n, c) for n in range(NL) for c in range(CB)]
            vblocks = blocks[:2]
            sblocks = blocks[2:]

            def emit_out_dma(n, cob, r0, r1):
                nc.sync.dma_start(
                    out=out_d[n, cob * 128:(cob + 1) * 128, r0:r1, :],
                    in_=xs4[cob][:, n, r0:r1, :])

            HALF = ((0, H // 2), (H // 2, H))
            # block 0: halves, pure Vector, immediately
            n0, c0 = vblocks[0]
            for r0, r1 in HALF:
                nc.vector.tensor_scalar(
                    out=y16[c0][:, n0, r0:r1, :], in0=y16[c0][:, n0, r0:r1, :],
                    scalar1=a2[:, c0:c0 + 1], scalar2=b2[:, c0:c0 + 1],
                    op0=ALU.mult, op1=ALU.add)
                nc.vector.tensor_add(
                    xs4[c0][:, n0, r0:r1, :], y16[c0][:, n0, r0:r1, :],
                    xs4[c0][:, n0, r0:r1, :])
                nc.vector.tensor_scalar(
                    out=xs4[c0][:, n0, r0:r1, :], in0=xs4[c0][:, n0, r0:r1, :],
                    scalar1=1.0, scalar2=-1.0, op0=ALU.min, op1=ALU.max)
                emit_out_dma(n0, c0, r0, r1)
            # scalar affines queue up behind a2/b2
            for n, cob in sblocks:
                nc.scalar.activation(
                    out=y16[cob][:, n, :, :], in_=y16[cob][:, n, :, :],
                    func=AF.Identity,
                    scale=a2[:, cob:cob + 1], bias=b2[:, cob:cob + 1])
            # block 1: whole-block Vector path
            n1, c1 = vblocks[1]
            nc.vector.tensor_scalar(
                out=y16[c1][:, n1, :, :], in0=y16[c1][:, n1, :, :],
                scalar1=a2[:, c1:c1 + 1], scalar2=b2[:, c1:c1 + 1],
                op0=ALU.mult, op1=ALU.add)
            nc.vector.tensor_add(
                xs4[c1][:, n1, :, :], y16[c1][:, n1, :, :],
                xs4[c1][:, n1, :, :])
            nc.vector.tensor_scalar(
                out=xs4[c1][:, n1, :, :], in0=xs4[c1][:, n1, :, :],
                scalar1=1.0, scalar2=-1.0, op0=ALU.min, op1=ALU.max)
            emit_out_dma(n1, c1, 0, H)

            for bi, (n, cob) in enumerate(sblocks):
                nc.vector.tensor_add(
                    xs4[cob][:, n, :, :], y16[cob][:, n, :, :],
                    xs4[cob][:, n, :, :])
                if bi == len(sblocks) - 1:
                    for r0, r1 in HALF:
                        nc.vector.tensor_scalar(
                            out=xs4[cob][:, n, r0:r1, :],
                            in0=xs4[cob][:, n, r0:r1, :],
                            scalar1=1.0, scalar2=-1.0,
                            op0=ALU.min, op1=ALU.max)
                        emit_out_dma(n, cob, r0, r1)
                else:
                    clip_eng = nc.gpsimd if bi < 3 else nc.vector
                    clip_eng.tensor_scalar(
                        out=xs4[cob][:, n, :, :], in0=xs4[cob][:, n, :, :],
                        scalar1=1.0, scalar2=-1.0,
                        op0=ALU.min, op1=ALU.max)
                    emit_out_dma(n, cob, 0, H)

    nc.compile()
    return nc


def _prep_inputs(x, w1, g1, b1, w2, g2, b2):
    """Host-side sharding + weight layout. Returns per-core input maps."""
    x = np.ascontiguousarray(np.asarray(x, dtype=np.float32))
    # fp16 x halves the HBM load traffic.  The residual add tolerates the
    # ~1e-3 rounding, and sign(x) is made exact by nudging the rare values
    # that would round to fp16 zero up to the smallest normal (sign kept).
    x16 = x.astype(np.float16)
    tiny = (x16 == 0) & (x != 0)
    if tiny.any():
        x16[tiny] = (np.sign(x[tiny]) * 6.104e-5).astype(np.float16)

    # sign(w) pre-transposed to the SBUF layout [ci%128, dy, dx, ci//128, co]
    def prep_w(w):
        wt = np.sign(np.asarray(w, np.float32)).transpose(1, 2, 3, 0)  # ci dy dx co
        wt = wt.reshape(2, 128, 3, 3, C).transpose(1, 2, 3, 0, 4)      # p dy dx k co
        return np.ascontiguousarray(wt).astype(NP_FP8)

    w1t = prep_w(w1)
    w2t = prep_w(w2)
    gb = np.stack(
        [np.asarray(v, np.float32)[c * 128:(c + 1) * 128]
         for v in (g1, b1, g2, b2) for c in range(CB)],
        axis=1,
    )
    # column order: g1_0 g1_1 b1_0 b1_1 g2_0 g2_1 b2_0 b2_1
    gb = np.ascontiguousarray(gb)
    in_maps = []
    for c in range(N_CORES):
        in_maps.append({
            "x16": np.ascontiguousarray(x16[c * NL:(c + 1) * NL]),
            "w1t": w1t,
            "w2t": w2t,
            "gb": gb,
        })
    return in_maps


def run(inputs, trace=False):
    """Run the kernel on 8 cores; returns (full_output, BassKernelResults)."""
    if "nc" not in _CACHE:
        _CACHE["nc"] = build()
    nc = _CACHE["nc"]
    in_maps = _prep_inputs(**inputs)
    res = bass_utils.run_bass_kernel_spmd(
        nc, in_maps, core_ids=list(range(N_CORES)), trace=trace)
    out = np.concatenate(
        [res.results[c]["out"].astype(np.float32) for c in range(N_CORES)], axis=0)
    return out, res


def kernel(**inputs):
    out, _ = run(inputs, trace=False)
    return out


# revision 41
# speedup vs baseline: 1.1386x; 1.1386x over previous
"""Trainium2 Bass kernel for a binarized (1w1a) BasicBlock:

    out1 = hardtanh(BN1(binconv(x, w1)))          # BN in training mode (batch stats)
    out  = hardtanh(BN2(binconv(out1, w2)) + x)   # identity shortcut

binconv(x, w) = conv3x3(sign(x), sign(w), pad=1) * (SCALE / K)

Sharding: data-parallel over batch (4 images per core on 8 cores), weights
replicated.  BN batch statistics (per-channel sum / sum-of-squares) are
exchanged with a tiny cross-core AllGather + local reduce (AllGather has a
~2x lower latency floor than AllReduce on an 8-core ring); a same-sized
dummy AllGather at kernel start absorbs the cold-plan setup cost.

Implementation notes (rewritten from the 349us baseline; ~310-320us):
  - sign() values are exact in fp8; conv3x3 = 9 shifted fp8 DoubleRow
    matmuls accumulating exact integers in fp32 PSUM.  SCALE/K is folded
    into BN eps (eps_eff = eps/(SCALE/K)^2).
  - 4-up packed activations: all 4 local images side by side in one padded
    row of pitch 240 ([z img(56) z] x4 + 8 pad cols, all offsets
    16B-aligned for DoubleRow), so one matmul covers 2 output rows x 4
    images (free dim 480/512, 93.3% useful vs 87.5% for a per-image
    64-pitch layout); each conv is 28 tiles x 2 channel blocks x 9 MMs.
  - The packed buffer is split into FOUR row-range tiles (rows 0-7 / 6-25
    / 22-41 / 38-57).  Tile tracks dependencies as byte intervals, and a
    matmul's rhs interval spans both channel blocks of its buffer, so any
    sign write into the same buffer would serialize against it; with four
    buffers the sign block for range k+1 runs fully under the matmuls of
    range k.  Both convs reuse the same four buffers.
  - PSUM evacuation: VectorE tensor_scalar copies to fp16 y16 tiles
    ([128, n, h, w]: contiguous per-image blocks for the final phase);
    bn_stats takes one 224-element record per (2 tiles, image); a single
    bn_aggr over tiles 0-25 is issued under the conv tail so only the
    last pair's records + aggregation are exposed at each BN boundary.
  - sign2 for conv2 is sign(A1*y1 + B1) fused into one ScalarE activation
    per (buffer, channel block) with per-partition scale/bias.
  - final phase out = clip(A2*y2 + B2 + x, -1, 1) is computed in place in
    the resident fp16 x tiles and is output-DMA-bound (6.4 MB ~ 18us):
    block 0 runs an all-Vector path in half-blocks so DMAs start ~2us
    after A2/B2; ScalarE affines 6 blocks in place in y16; GpSimd
    (pre-warmed mid-conv2) takes 3 middle clips; output DMAs ride the
    sync queue so the gpsimd software-DGE drain stays short.
  - startup: x rows [0:7) of all images land first (one contiguous DMA
    per (image, channel block)), 8 junk matmuls warm the PE HAM clock
    gate while they're in flight, and the first sign covers only rows
    0-2 so the first conv matmul issues at ~15us.
"""

import numpy as np
import ml_dtypes

import concourse.tile as tile
from concourse import bacc, mybir
from concourse import bass_utils

N_CORES = 8
N, C, H, W = 32, 256, 56, 56
NL = N // N_CORES          # images per core (4)
CB = C // 128              # channel blocks (2)
HP = H + 2                 # padded rows (58)
IW = 58                    # per-image slot in a packed row (halo+56+halo share)
RP = 240                   # packed row pitch: 4*58 + 8 pad, 16B-aligned
BLK = HP * RP              # per-channel-block packed image elements (13920)
HT = 2                     # output rows per tile
NT = H // HT               # 28 tiles per conv per channel block
FREE = HT * RP             # 480 free-dim per matmul
SCALE, K = 1.0, 2
EPS = 1e-5
ALPHA = SCALE / K
EPS_EFF = EPS / (ALPHA * ALPHA)
M_TOT = float(N * H * W)   # BN reduction count (global batch)

F32 = mybir.dt.float32
F16 = mybir.dt.float16
FP8 = mybir.dt.float8e4
NP_FP8 = ml_dtypes.float8_e4m3
AF = mybir.ActivationFunctionType
ALU = mybir.AluOpType
DR = mybir.MatmulPerfMode.DoubleRow

_CACHE = {}
DEBUG_DUMPS = False


def _emit_stats_convert(nc, small, mv_pair, m_loc, st, tagp):
    """(mean, var) pair per cob -> packed (sum, ssum) into st[:, 0:4]."""
    for cob in range(CB):
        mv = mv_pair[cob]
        tmp = small.tile([128, 1], F32, tag=f"cv{tagp}{cob}", name=f"cv{tagp}{cob}")
        nc.vector.tensor_scalar_mul(st[:, 2 * cob:2 * cob + 1], mv[:, 0:1], m_loc)
        nc.vector.tensor_mul(tmp[:], mv[:, 0:1], mv[:, 0:1])
        nc.vector.tensor_add(tmp[:], tmp[:], mv[:, 1:2])
        nc.vector.tensor_scalar_mul(st[:, 2 * cob + 1:2 * cob + 2], tmp[:], m_loc)


def _emit_affine(nc, small, gsum, gb, g_col, epst, a_t, b_t):
    """A = g*rsqrt(var+eps_eff), B = b - mean*A, both cobs batched [128,2].

    gsum: [128, 4] globally-reduced (s0, ss0, s1, ss1)
    gb column layout: g1_0 g1_1 b1_0 b1_1 g2_0 g2_1 b2_0 b2_1
    """
    g4 = gsum[:].rearrange("p (c s) -> p s c", s=2)  # [:,0,:]=sums, [:,1,:]=ssums
    mean2 = small.tile([128, 2], F32, tag="af_mean", name="af_mean")
    ex2 = small.tile([128, 2], F32, tag="af_ex2", name="af_ex2")
    var2 = small.tile([128, 2], F32, tag="af_var", name="af_var")
    rstd = small.tile([128, 2], F32, tag="af_rstd", name="af_rstd")
    nc.vector.tensor_scalar_mul(mean2[:], g4[:, 0, :], 1.0 / M_TOT)
    nc.vector.tensor_scalar_mul(ex2[:], g4[:, 1, :], 1.0 / M_TOT)
    nc.vector.tensor_mul(var2[:], mean2[:], mean2[:])
    nc.vector.tensor_sub(var2[:], ex2[:], var2[:])
    nc.scalar.activation(out=rstd[:], in_=var2[:], func=AF.Sqrt, bias=epst[:])
    nc.vector.reciprocal(rstd[:], rstd[:])
    nc.vector.tensor_mul(a_t[:], gb[:, g_col:g_col + 2], rstd[:])
    nc.vector.tensor_mul(mean2[:], mean2[:], a_t[:])
    nc.vector.tensor_sub(b_t[:], gb[:, g_col + 2:g_col + 4], mean2[:])


def build():
    """Build + compile the per-core Bass program (SPMD, 8 cores)."""
    nc = bacc.Bacc("TRN2", target_bir_lowering=False, debug=False,
                   num_devices=N_CORES)

    x_in = nc.dram_tensor("x16", [NL, C, H, W], F16, kind="ExternalInput").ap()
    w1_in = nc.dram_tensor("w1t", [128, 3, 3, 2, C], FP8, kind="ExternalInput").ap()
    w2_in = nc.dram_tensor("w2t", [128, 3, 3, 2, C], FP8, kind="ExternalInput").ap()
    gb_in = nc.dram_tensor("gb", [128, 8], F32, kind="ExternalInput").ap()
    out_d = nc.dram_tensor("out", [NL, C, H, W], F16, kind="ExternalOutput").ap()
    dbg = {}
    if DEBUG_DUMPS:
        dbg["xb"] = nc.dram_tensor("dbg_xb", [128, CB, BLK], FP8,
                                   kind="ExternalOutput").ap()
        dbg["xb2"] = nc.dram_tensor("dbg_xb2", [128, CB, BLK], FP8,
                                    kind="ExternalOutput").ap()
        for c in range(CB):
            dbg[f"y1_{c}"] = nc.dram_tensor(f"dbg_y1_{c}", [128, NL, H, W], F16,
                                            kind="ExternalOutput").ap()
            dbg[f"y2_{c}"] = nc.dram_tensor(f"dbg_y2_{c}", [128, NL, H, W], F16,
                                            kind="ExternalOutput").ap()
        dbg["st1"] = nc.dram_tensor("dbg_st1", [128, 4], F32,
                                    kind="ExternalOutput").ap()
        dbg["g81"] = nc.dram_tensor("dbg_g81", [128, N_CORES, 4], F32,
                                    kind="ExternalOutput").ap()
        dbg["ab1"] = nc.dram_tensor("dbg_ab1", [128, 4], F32,
                                    kind="ExternalOutput").ap()
        dbg["ab2"] = nc.dram_tensor("dbg_ab2", [128, 4], F32,
                                    kind="ExternalOutput").ap()

    rg = [list(range(N_CORES))]

    with tile.TileContext(nc) as tc:
        import contextlib
        with contextlib.ExitStack() as ctx:
            consts = ctx.enter_context(tc.tile_pool(name="consts", bufs=1))
            xbp = ctx.enter_context(tc.tile_pool(name="xbp", bufs=1))
            y16p = ctx.enter_context(tc.tile_pool(name="y16p", bufs=1))
            xsp = ctx.enter_context(tc.tile_pool(name="xsp", bufs=1))
            statp = ctx.enter_context(tc.tile_pool(name="statp", bufs=1))
            small = ctx.enter_context(tc.tile_pool(name="small", bufs=1))
            psum = ctx.enter_context(tc.tile_pool(name="psum", bufs=8, space="PSUM"))
            dram = ctx.enter_context(tc.tile_pool(name="dram", bufs=1, space="DRAM"))

            # ---- junk-matmul HAM warmup material (emitted first so the PE
            # runs while the first x stripes are still in flight)
            jw = consts.tile([128, 128], FP8, tag="jw", name="jw")
            jx = consts.tile([128, FREE], FP8, tag="jx", name="jx")
            nc.vector.memset(jw[:].bitcast(mybir.dt.uint32), 0)
            nc.vector.memset(jx[:].bitcast(mybir.dt.uint32), 0)
            gs_warm = consts.tile([128, 64], F16, tag="gsw", name="gsw")
            nc.vector.memset(gs_warm[:].bitcast(mybir.dt.uint32), 0)
            jp = psum.tile([128, FREE + 2], F32, tag="pt", name="jp")
            for _ in range(8):
                nc.tensor.matmul(jp[:, 0:FREE], jw[:], jx[:], start=True, stop=True)

            # ---- staged fp16 x per channel block; doubles as the residual
            # and (in place) the final output buffer.
            xs4 = [xsp.tile([128, NL, H, W], F16, tag=f"xs4_{c}", name=f"xs4_{c}")
                   for c in range(CB)]

            def load_x(cib, n, r0, r1, q):
                # contiguous per-image chunks: 128 descriptors, fast desc-gen
                q.dma_start(out=xs4[cib][:, n, r0:r1, :],
                            in_=x_in[n, cib * 128:(cib + 1) * 128, r0:r1, :])

            R_SM, R_MID = 7, 25
            # first stripes rows [0:7) gate the first sign block: w1 + n0/n1
            # on sync, n2/n3 on gpsimd (before the collective plumbing)
            for n in range(NL):
                for cib in range(CB):
                    load_x(cib, n, 0, R_SM, nc.sync)
            w1t = consts.tile([128, 3, 3, 2, C], FP8, tag="w1t", name="w1t")
            nc.sync.dma_start(out=w1t[:], in_=w1_in[:])
            for n in range(NL):
                for cib in range(CB):
                    load_x(cib, n, R_SM, R_MID, nc.sync)
            # tail chunks are enqueued mid-conv1 (pre_tile) so the sign
            # blocks emitted before them don't pick up a false WAR on them

            # ---- packed binarized activations: FOUR row-range buffers so
            # sign writes for range k+1 never alias the buffer the PE is
            # currently reading (Tile tracks deps as byte intervals; one
            # matmul's rhs interval spans both channel blocks of a buffer,
            # so any in-buffer sign write would serialize with it).
            # Padded-row ranges: A 0..7 (tiles 0-2), B 6..25 (tiles 3-11),
            # C 22..41 (tiles 12-19), D 38..57 (tiles 20-27).
            XB_BASE = (0, 6, 22, 38)
            XB_ROWS = (8, 20, 20, 20)
            XB_OF_T = lambda t: 0 if t <= 2 else (1 if t <= 11 else (2 if t <= 19 else 3))
            xbufs = [xbp.tile([128, CB, r * RP], FP8, tag=f"xb{i}", name=f"xb{i}")
                     for i, r in enumerate(XB_ROWS)]

            def memset_halo(bi, q):
                """zero halo cols + pitch padding (+ halo rows in A/D)."""
                rows_b = XB_ROWS[bi]
                for cib in range(CB):
                    rows = xbufs[bi][:, cib, :].rearrange("p (r c) -> p r c", c=RP)
                    q.memset(rows[:, :, NL * IW:RP].bitcast(mybir.dt.uint32), 0)
                    imgs = rows[:, :, 0:NL * IW].rearrange(
                        "p r (n c) -> p r n c", c=IW)
                    q.memset(imgs[:, :, :, 0:1], 0.0)
                    q.memset(imgs[:, :, :, IW - 1:IW], 0.0)
                    if bi == 0:
                        q.memset(rows[:, 0, :], 0.0)
                    if bi == 3:
                        q.memset(rows[:, rows_b - 1, :], 0.0)

            memset_halo(0, nc.vector)
            memset_halo(1, nc.vector)   # B is needed by ~t=27us; vector is fast

            epst = small.tile([128, 1], F32, tag="epst", name="epst")
            nc.vector.memset(epst[:], EPS_EFF)

            # ---- dummy AllGather: absorb first-collective setup cost
            dzero = small.tile([128, 4], F32, tag="dzero", name="dzero")
            nc.vector.memset(dzero[:], 0.0)
            d_in0 = dram.tile([128, 4], F32, tag="d_in0", name="d_in0")
            d_out0 = dram.tile([N_CORES, 128, 4], F32, tag="d_out0", name="d_out0")
            nc.gpsimd.dma_start(out=d_in0[:], in_=dzero[:])
            nc.gpsimd.collective_compute(
                "AllGather", ALU.bypass, replica_groups=rg,
                ins=[d_in0.opt()], outs=[d_out0.opt()],
            )
            for bi in (2, 3):
                memset_halo(bi, nc.gpsimd)

            # ---- remaining constants
            w2t = consts.tile([128, 3, 3, 2, C], FP8, tag="w2t", name="w2t")
            nc.sync.dma_start(out=w2t[:], in_=w2_in[:])
            gb = consts.tile([128, 8], F32, tag="gb", name="gb")
            nc.sync.dma_start(out=gb[:], in_=gb_in[:])

            # ---- conv outputs as exact integers (reused conv1 -> conv2)
            y16 = [y16p.tile([128, NL, H, W], F16, tag=f"y16_{c}", name=f"y16_{c}")
                   for c in range(CB)]

            # one record per (2 tiles, image): 224-elem contiguous runs.
            # record stride padded to 8 so the 6-wide record dim can't be
            # AP-merged away (the BIR verifier wants innermost extent 6)
            recs = [statp.tile([128, NT // 2, NL, 8], F32, tag=f"rec{c}",
                               name=f"rec{c}") for c in range(CB)]

            # sign blocks: per buffer, interior image rows it must carry
            #   A: img 0..6   B: img 5..24   C: img 21..40   D: img 37..55
            SIGN_RANGES = ((0, 7), (5, 25), (21, 41), (37, 56))

            def sign_block(bi, a_t=None, b_t=None, rr=None):
                """sign into buffer bi: conv1 (from xs4) or conv2 (from y16
                with BN affine) depending on a_t."""
                base = XB_BASE[bi]
                r0, r1 = rr if rr is not None else SIGN_RANGES[bi]
                for cib in range(CB):
                    rows = xbufs[bi][:, cib, :].rearrange(
                        "p (r c) -> p r c", c=RP)
                    dst = (rows[:, :, 0:NL * IW]
                           .rearrange("p r (n c) -> p n r c", c=IW)
                           [:, :, 1 + r0 - base:1 + r1 - base, 1:W + 1])
                    if a_t is None:
                        nc.scalar.activation(
                            out=dst, in_=xs4[cib][:, :, r0:r1, :], func=AF.Sign)
                    else:
                        nc.scalar.activation(
                            out=dst, in_=y16[cib][:, :, r0:r1, :], func=AF.Sign,
                            scale=a_t[:, cib:cib + 1], bias=b_t[:, cib:cib + 1])

            def conv(wt, pre_tile, tagp):
                """One binarized conv3x3 over the 4 packed row-range buffers.

                pre_tile: dict tile_idx -> emit callbacks (sign blocks for
                upcoming buffers, lazy DMA enqueues...).
                Returns per-cob (mean, var) tiles for the local batch."""
                mv = [small.tile([128, 2], F32, tag=f"mv{tagp}{c}", name=f"mv{tagp}{c}")
                      for c in range(CB)]
                mvB = [small.tile([128, 2], F32, tag=f"mvB{tagp}{c}",
                                  name=f"mvB{tagp}{c}") for c in range(CB)]
                for t in range(NT):
                    for fn in pre_tile.get(t, ()):
                        fn()
                    if t == NT - 1:
                        for cob in range(CB):
                            rv = recs[cob][:, 0:NT // 2 - 1, :, 0:6].rearrange(
                                "p t n s -> p (t n) s")
                            nc.vector.bn_aggr(out=mv[cob][:], in_=rv)
                    h0 = t * HT
                    bi = XB_OF_T(t)
                    base = XB_BASE[bi]
                    src = xbufs[bi][:]
                    for cob in range(CB):
                        pt = psum.tile([128, FREE + 2], F32, tag="pt", name="pt")
                        k = 0
                        for dy in range(3):
                            off = (h0 + dy - base) * RP
                            for dx in range(3):
                                nc.tensor.matmul(
                                    pt[:, 2 - dx:FREE + 2 - dx],
                                    wt[:, dy, dx, :, cob * 128:(cob + 1) * 128],
                                    src[:, :, off:off + FREE],
                                    start=(k == 0),
                                    stop=(k == 8),
                                    perf_mode=DR,
                                )
                                k += 1
                        # useful output view, iterated (image, row, col).
                        # out pixel w of image n sits at window offset
                        # r*RP + n*IW + w (NOT +1: the dx-shifted windows
                        # already center the 3x3 stencil)
                        ptv = (pt[:, 2:FREE + 2]
                               .rearrange("p (r c) -> p r c", c=RP)[:, :, 0:NL * IW]
                               .rearrange("p r (n c) -> p n r c", c=IW)[:, :, :, 0:W])
                        nc.vector.tensor_scalar_mul(
                            y16[cob][:, :, h0:h0 + HT, :], ptv, 1.0)
                        if t % 2 == 1:
                            # single-record bn_stats per (2 tiles, image)
                            for n in range(NL):
                                nc.vector.bn_stats(
                                    out=recs[cob][:, t // 2, n, 0:6],
                                    in_=y16[cob][:, n, h0 - HT:h0 + HT, :]
                                    .rearrange("p r c -> p (r c)"))
                for cob in range(CB):
                    nc.vector.bn_aggr(out=mvB[cob][:],
                                      in_=recs[cob][:, NT // 2 - 1, :, 0:6])
                return mv, mvB

            def stats_gather(mvs, tagp):
                """Pack local chunked (mean,var) -> (sum, ssum)[128,4],
                AllGather, reduce the 8 cores' contributions locally."""
                mv, mvB = mvs
                stA = small.tile([128, 4], F32, tag=f"stA{tagp}", name=f"stA{tagp}")
                stB = small.tile([128, 4], F32, tag=f"stB{tagp}", name=f"stB{tagp}")
                _emit_stats_convert(nc, small, mv, float(26 * NL * HT * W),
                                    stA, tagp)
                _emit_stats_convert(nc, small, mvB, float(2 * NL * HT * W),
                                    stB, "b" + tagp)
                nc.vector.tensor_add(stA[:], stA[:], stB[:])
                d_in = dram.tile([128, 4], F32, tag=f"din{tagp}", name=f"din{tagp}")
                d_out = dram.tile([N_CORES, 128, 4], F32, tag=f"dou{tagp}",
                                  name=f"dou{tagp}")
                nc.gpsimd.dma_start(out=d_in[:], in_=stA[:])
                nc.gpsimd.collective_compute(
                    "AllGather", ALU.bypass, replica_groups=rg,
                    ins=[d_in.opt()], outs=[d_out.opt()],
                )
                g8 = small.tile([128, N_CORES, 4], F32, tag=f"g8{tagp}",
                                name=f"g8{tagp}")
                nc.gpsimd.dma_start(out=g8[:],
                                    in_=d_out[:].rearrange("r p c -> p r c"))
                t4 = small.tile([128, 4, 4], F32, tag=f"t4{tagp}", name=f"t4{tagp}")
                t2 = small.tile([128, 2, 4], F32, tag=f"t2{tagp}", name=f"t2{tagp}")
                gs = small.tile([128, 4], F32, tag=f"gs{tagp}", name=f"gs{tagp}")
                nc.vector.tensor_add(t4[:], g8[:, 0:4, :], g8[:, 4:8, :])
                nc.vector.tensor_add(t2[:], t4[:, 0:2, :], t4[:, 2:4, :])
                nc.vector.tensor_add(gs[:].rearrange("p (o c) -> p o c", o=1),
                                     t2[:, 0:1, :], t2[:, 1:2, :])
                if DEBUG_DUMPS and tagp == "1":
                    nc.scalar.dma_start(out=dbg["st1"], in_=stA[:])
                    nc.scalar.dma_start(out=dbg["g81"], in_=g8[:])
                return gs

            def tails_enq():
                for n in range(NL):
                    for cib in range(CB):
                        load_x(cib, n, R_MID, H, nc.sync)

            # ================= conv1 =================
            sign_block(0, rr=(0, 3))     # just enough for conv tile 0
            pre1 = {
                1: [lambda: sign_block(0, rr=(3, 7))],
                2: [lambda: sign_block(1)],
                4: [tails_enq],
                6: [lambda: sign_block(2)],
                12: [lambda: sign_block(3)],
            }
            mv1 = conv(w1t, pre1, "1")  # -> (mvA, mvB)
            if DEBUG_DUMPS:
                for c in range(CB):
                    nc.scalar.dma_start(out=dbg[f"y1_{c}"], in_=y16[c][:])
            gs1 = stats_gather(mv1, "1")
            a1 = small.tile([128, 2], F32, tag="a1", name="a1")
            b1 = small.tile([128, 2], F32, tag="b1", name="b1")
            _emit_affine(nc, small, gs1, gb, 0, epst, a1, b1)
            if DEBUG_DUMPS:
                nc.scalar.dma_start(out=dbg["ab1"][:, 0:2], in_=a1[:])
                nc.scalar.dma_start(out=dbg["ab1"][:, 2:4], in_=b1[:])

            # ================= conv2 =================
            sign_block(0, a1, b1, rr=(0, 3))
            pre2 = {
                1: [lambda: sign_block(0, a1, b1, rr=(3, 7))],
                2: [lambda: sign_block(1, a1, b1)],
                5: [lambda: sign_block(2, a1, b1)],
                12: [lambda: sign_block(3, a1, b1)],
                20: [lambda: nc.gpsimd.tensor_scalar(
                    out=gs_warm[:], in0=gs_warm[:],
                    scalar1=1.0, scalar2=-1.0, op0=ALU.min, op1=ALU.max)],
            }
            mv2 = conv(w2t, pre2, "2")
            if DEBUG_DUMPS:
                for c in range(CB):
                    nc.scalar.dma_start(out=dbg[f"y2_{c}"], in_=y16[c][:])
            gs2 = stats_gather(mv2, "2")
            a2 = small.tile([128, 2], F32, tag="a2", name="a2")
            b2 = small.tile([128, 2], F32, tag="b2", name="b2")
            _emit_affine(nc, small, gs2, gb, 4, epst, a2, b2)
            if DEBUG_DUMPS:
                nc.scalar.dma_start(out=dbg["ab2"][:, 0:2], in_=a2[:])
                nc.scalar.dma_start(out=dbg["ab2"][:, 2:4], in_=b2[:])

            # ================= final =================
            # out = clip(A2*y2 + B2 + x, -1, 1), computed in place in xs4.
            # Output-DMA-bound phase (6.4 MB fp16 ~ 18us): block 0 runs the
            # all-Vector path in half-blocks so DMAs start ~2us after A2/B2;
            # ScalarE affines 6 blocks; GpSimd (pre-warmed) takes 3 middle
            # clips; the last block is halved to shorten the tail.
            blocks = [(n, c) for n in range(NL) for c in range(CB)]
            vblocks = blocks[:2]
            sblocks = blocks[2:]

            def emit_out_dma(n, cob, r0, r1):
                nc.sync.dma_start(
                    out=out_d[n, cob * 128:(cob + 1) * 128, r0:r1, :],
                    in_=xs4[cob][:, n, r0:r1, :])

            HALF = ((0, H // 2), (H // 2, H))
            # block 0: halves, pure Vector, immediately
            n0, c0 = vblocks[0]
            for r0, r1 in HALF:
                nc.vector.tensor_scalar(
                    out=y16[c0][:, n0, r0:r1, :], in0=y16[c0][:, n0, r0:r1, :],
                    scalar1=a2[:, c0:c0 + 1], scalar2=b2[:, c0:c0 + 1],
                    op0=ALU.mult, op1=ALU.add)
                nc.vector.tensor_add(
                    xs4[c0][:, n0, r0:r1, :], y16[c0][:, n0, r0:r1, :],
                    xs4[c0][:, n0, r0:r1, :])
                nc.vector.tensor_scalar(
                    out=xs4[c0][:, n0, r0:r1, :], in0=xs4[c0][:, n0, r0:r1, :],
                    scalar1=1.0, scalar2=-1.0, op0=ALU.min, op1=ALU.max)
                emit_out_dma(n0, c0, r0, r1)
            # scalar affines queue up behind a2/b2
            for n, cob in sblocks:
                nc.scalar.activation(
                    out=y16[cob][:, n, :, :], in_=y16[cob][:, n, :, :],
                    func=AF.Identity,
                    scale=a2[:, cob:cob + 1], bias=b2[:, cob:cob + 1])
            # block 1: whole-block Vector path
            n1, c1 = vblocks[1]
            nc.vector.tensor_scalar(
                out=y16[c1][:, n1, :, :], in0=y16[c1][:, n1, :, :],
                scalar1=a2[:, c1:c1 + 1], scalar2=b2[:, c1:c1 + 1],
                op0=ALU.mult, op1=ALU.add)
            nc.vector.tensor_add(
                xs4[c1][:, n1, :, :], y16[c1][:, n1, :, :],
                xs4[c1][:, n1, :, :])
            nc.vector.tensor_scalar(
                out=xs4[c1][:, n1, :, :], in0=xs4[c1][:, n1, :, :],
                scalar1=1.0, scalar2=-1.0, op0=ALU.min, op1=ALU.max)
            emit_out_dma(n1, c1, 0, H)

            for bi, (n, cob) in enumerate(sblocks):
                nc.vector.tensor_add(
                    xs4[cob][:, n, :, :], y16[cob][:, n, :, :],
                    xs4[cob][:, n, :, :])
                if bi == len(sblocks) - 1:
                    for r0, r1 in HALF:
                        nc.vector.tensor_scalar(
                            out=xs4[cob][:, n, r0:r1, :],
                            in0=xs4[cob][:, n, r0:r1, :],
                            scalar1=1.0, scalar2=-1.0,
                            op0=ALU.min, op1=ALU.max)
                        emit_out_dma(n, cob, r0, r1)
                else:
                    clip_eng = nc.gpsimd if bi < 3 else nc.vector
                    clip_eng.tensor_scalar(
                        out=xs4[cob][:, n, :, :], in0=xs4[cob][:, n, :, :],
                        scalar1=1.0, scalar2=-1.0,
                        op0=ALU.min, op1=ALU.max)
                    emit_out_dma(n, cob, 0, H)

    nc.compile()
    return nc


def _prep_inputs(x, w1, g1, b1, w2, g2, b2):
    """Host-side sharding + weight layout. Returns per-core input maps."""
    x = np.ascontiguousarray(np.asarray(x, dtype=np.float32))
    # fp16 x halves the HBM load traffic.  The residual add tolerates the
    # ~1e-3 rounding, and sign(x) is made exact by nudging the rare values
    # that would round to fp16 zero up to the smallest normal (sign kept).
    x16 = x.astype(np.float16)
    tiny = (x16 == 0) & (x != 0)
    if tiny.any():
        x16[tiny] = (np.sign(x[tiny]) * 6.104e-5).astype(np.float16)

    # sign(w) pre-transposed to the SBUF layout [ci%128, dy, dx, ci//128, co]
    def prep_w(w):
        wt = np.sign(np.asarray(w, np.float32)).transpose(1, 2, 3, 0)  # ci dy dx co
        wt = wt.reshape(2, 128, 3, 3, C).transpose(1, 2, 3, 0, 4)      # p dy dx k co
        return np.ascontiguousarray(wt).astype(NP_FP8)

    w1t = prep_w(w1)
    w2t = prep_w(w2)
    gb = np.stack(
        [np.asarray(v, np.float32)[c * 128:(c + 1) * 128]
         for v in (g1, b1, g2, b2) for c in range(CB)],
        axis=1,
    )
    # column order: g1_0 g1_1 b1_0 b1_1 g2_0 g2_1 b2_0 b2_1
    gb = np.ascontiguousarray(gb)
    in_maps = []
    for c in range(N_CORES):
        in_maps.append({
            "x16": np.ascontiguousarray(x16[c * NL:(c + 1) * NL]),
            "w1t": w1t,
            "w2t": w2t,
            "gb": gb,
        })
    return in_maps


def run(inputs, trace=False):
    """Run the kernel on 8 cores; returns (full_output, BassKernelResults)."""
    if "nc" not in _CACHE:
        _CACHE["nc"] = build()
    nc = _CACHE["nc"]
    in_maps = _prep_inputs(**inputs)
    res = bass_utils.run_bass_kernel_spmd(
        nc, in_maps, core_ids=list(range(N_CORES)), trace=trace)
    out = np.concatenate(
        [res.results[c]["out"].astype(np.float32) for c in range(N_CORES)], axis=0)
    return out, res


def kernel(**inputs):
    out, _ = run(inputs, trace=False)
    return out
